# revision 1
# baseline (speedup 1.0000x reference)
"""Trainium2 Bass kernel for nn_CausalSelfAttention_18519898980537.

Low-rank causal self-attention (12 heads, head_dim 64, only the first 16
query dims enter the scores; latent k/v dim 16/head), with the
reference's transpose-reshape scramble before the output projection.

Sharding (8 cores, SPMD single program, per-core differences are input
data only): core c = 2*b + p handles batch b with heads [6p, 6p+6) over
the full causal range.  Per query block, the normalized attention
output alT [2x96, 512] is exchanged pairwise via AllGather so both
cores of a pair see all 12 heads; each core then computes
output-feature half p of ao^T = Wvu^T @ alT, writes it to a DRAM
scratch whose flat reinterpretation is exactly the reference's
reshape(transpose(ao)) for its token half, PE-transposes the re-read
tiles, and applies its token-half of the output projection.

Projections and the final matmuls run in float32r (TF32-like, 1
cycle/row at N>=256); attention (scores and value matmuls) runs in bf16
with row/column tile packing.  Softmax needs no max-subtraction (logits
are O(1) by construction); normalization happens after the value matmul
via an appended ones-column that accumulates the denominators.
"""

import os
import sys

import numpy as np

for _p in ("/opt/trn_rl_repo", "/root/.axon_site/_ro/trn_rl_repo"):
    if os.path.isdir(_p) and _p not in sys.path:
        sys.path.insert(0, _p)

import ml_dtypes  # noqa: E402
import concourse.bacc as bacc  # noqa: E402
import concourse.mybir as mybir  # noqa: E402
from concourse.tile import TileContext  # noqa: E402
from concourse.bass_utils import run_bass_kernel_spmd  # noqa: E402

H, DH, LD, HID, LAT = 12, 64, 16, 768, 192
B, S = 4, 2048
N_CORES = 8
HL = 6  # heads per core
MHALF = HID // 2  # 384 = aoT rows per core
SCALE = LD ** -0.5

f32 = mybir.dt.float32
f32r = mybir.dt.float32r
bf16 = mybir.dt.bfloat16
AF = mybir.ActivationFunctionType
MULT = mybir.AluOpType.mult

_CACHE = {}


def _build_nc():
    if "nc" in _CACHE:
        return _CACHE["nc"]
    nc = bacc.Bacc("TRN2", target_bir_lowering=False, debug=False, num_devices=N_CORES)

    XT = nc.dram_tensor("xt", [HID + 1, S], f32r, kind="ExternalInput")
    WQ = nc.dram_tensor("wq", [HID + 1, 256], f32r, kind="ExternalInput")
    WK = nc.dram_tensor("wk", [HID + 1, 256], f32r, kind="ExternalInput")
    WV = nc.dram_tensor("wv", [HID + 1, 256], f32r, kind="ExternalInput")
    WVU = nc.dram_tensor("wvu", [384, MHALF], f32r, kind="ExternalInput")
    WO = nc.dram_tensor("wo", [HID + 1, HID], f32r, kind="ExternalInput")
    MSK = nc.dram_tensor("mask", [128, 128], bf16, kind="ExternalInput")
    IDN = nc.dram_tensor("ident", [128, 128], f32, kind="ExternalInput")
    EG = nc.dram_tensor("eg", [96, 3], f32r, kind="ExternalInput")
    EB = nc.dram_tensor("eb", [3, 96], f32r, kind="ExternalInput")
    ONES = nc.dram_tensor("ones", [1, 1024], f32r, kind="ExternalInput")
    OUTT = nc.dram_tensor("outt", [HID, S // 2], f32, kind="ExternalOutput")

    rg = [[0, 1], [2, 3], [4, 5], [6, 7]]

    with TileContext(nc) as tc:
        with (
            tc.tile_pool(name="const", bufs=1) as const,
            tc.tile_pool(name="act", bufs=1) as act,
            tc.tile_pool(name="work", bufs=1) as work,
            tc.tile_pool(name="ps", bufs=1, space="PSUM") as ps,
            tc.tile_pool(name="dram", bufs=1, space="DRAM") as dram,
        ):
            # ---- constant loads -------------------------------------------------
            def load_w(handle, ncols, nm):
                tiles = []
                for k in range(7):
                    pp = 128 if k < 6 else 1
                    t = const.tile(
                        [pp, ncols], f32r, name=f"{nm}{k}", tag="wmat", bufs=8,
                        padded_shape=[128, HID],
                    )
                    nc.sync.dma_start(out=t[:], in_=handle[128 * k : 128 * k + pp, :])
                    tiles.append(t)
                return tiles

            wq_sb = load_w(WQ, 256, "wq")
            wk_sb = load_w(WK, 256, "wk")
            wv_sb = load_w(WV, 256, "wv")

            xt_sb = []
            for k in range(7):
                pp = 128 if k < 6 else 1
                t = const.tile([pp, S], f32r, name=f"xt{k}")
                nc.sync.dma_start(out=t[:], in_=XT[128 * k : 128 * k + pp, :])
                xt_sb.append(t)

            msk_sb = const.tile([128, 128], bf16, name="msk")
            nc.sync.dma_start(out=msk_sb[:], in_=MSK[:, :])
            idn_sb = const.tile([128, 128], f32, name="idn")
            nc.sync.dma_start(out=idn_sb[:], in_=IDN[:, :])
            eg_sb = const.tile([96, 3], f32r, name="eg")
            nc.sync.dma_start(out=eg_sb[:], in_=EG[:, :])
            eb_sb = const.tile([3, 96], f32r, name="eb")
            nc.sync.dma_start(out=eb_sb[:], in_=EB[:, :])
            ones_row = const.tile([1, 1024], f32r, name="ones_row")
            nc.sync.dma_start(out=ones_row[:], in_=ONES[:, :])

            # ---- q/k projections (bf16 outputs feed the score matmuls) ---------
            qlT = [act.tile([128, S], bf16, name=f"qlT{t}") for t in range(2)]
            klT = [act.tile([128, S], bf16, name=f"klT{t}") for t in range(2)]
            for wt, dst in ((wq_sb, qlT), (wk_sb, klT)):
                for t in range(2):
                    for nb in range(4):
                        pt = ps.tile([128, 512], f32, tag="pg", bufs=1, name="projp")
                        for k in range(7):
                            nc.tensor.matmul(
                                pt[:],
                                wt[k][:, 128 * t : 128 * t + 128],
                                xt_sb[k][:, 512 * nb : 512 * nb + 512],
                                start=(k == 0),
                                stop=(k == 6),
                            )
                        nc.vector.tensor_copy(dst[t][:, 512 * nb : 512 * nb + 512], pt[:])

            vl_sb = [act.tile([128, 192], bf16, name=f"vl{i}") for i in range(16)]

            def vl_proj(i):
                pt = ps.tile([128, 256], f32, tag="pg", bufs=1, name="projv")
                for k in range(7):
                    nc.tensor.matmul(
                        pt[:],
                        xt_sb[k][:, 128 * i : 128 * i + 128],
                        wv_sb[k][:],
                        start=(k == 0),
                        stop=(k == 6),
                    )
                nc.vector.tensor_copy(vl_sb[i][:], pt[:, 0:192])

            for i in range(4):
                vl_proj(i)

            # ---- attention (software-pipelined over kt pairs) -------------------
            alt_t = [act.tile([96, S], f32r, name=f"alt{t}") for t in range(2)]
            cin = [dram.tile([192, 512], f32r, name=f"cin{c}") for c in range(4)]
            cout = [dram.tile([384, 512], f32r, name=f"cout{c}") for c in range(4)]
            wvu_sb = [const.tile([128, MHALF], f32r, name=f"wvu{k}") for k in range(3)]
            for k in range(3):
                nc.sync.dma_start(out=wvu_sb[k][:], in_=WVU[128 * k : 128 * k + 128, :])
            aot = dram.tile([MHALF, S], f32, name="aot")

            for qb in range(4):
                for t in range(2):
                    avp = ps.tile([128, 512], f32, tag="av", bufs=1, name="avp")
                    nkt = 4 * qb + 4
                    npair = nkt // 2
                    exs = {}

                    def emit_scores(j, t=t, qb=qb, exs=exs):
                        kts = (2 * j, 2 * j + 1)
                        offs = [
                            (128 * (kt - 4 * qb) if kt - 4 * qb > 0 else 0) for kt in kts
                        ]
                        for g in range(3):
                            scp = ps.tile([128, 1024], f32, tag="sc", bufs=3, name="scp")
                            for ii, kt in enumerate(kts):
                                off = offs[ii]
                                nc.tensor.matmul(
                                    scp[:, 512 * ii + off : 512 * ii + 512],
                                    klT[t][32 * g : 32 * g + 16, 128 * kt : 128 * kt + 128],
                                    qlT[t][
                                        32 * g : 32 * g + 16,
                                        512 * qb + off : 512 * qb + 512,
                                    ],
                                    start=True,
                                    stop=True,
                                    tile_position=(32 * g, 0),
                                )
                            ex = work.tile([128, 1024], bf16, tag="expT", bufs=6, name="ex")
                            nc.scalar.activation(
                                ex[:], scp[:], AF.Exp, bias=0.0, scale=SCALE
                            )
                            for ii, kt in enumerate(kts):
                                if kt - 4 * qb >= 0:
                                    off = offs[ii]
                                    nc.vector.tensor_tensor(
                                        ex[:, 512 * ii + off : 512 * ii + off + 128],
                                        ex[:, 512 * ii + off : 512 * ii + off + 128],
                                        msk_sb[:],
                                        op=MULT,
                                    )
                            exs[(j, g)] = ex

                    def emit_av(j, t=t, qb=qb, nkt=nkt, avp=avp, exs=exs):
                        kts = (2 * j, 2 * j + 1)
                        offs = [
                            (128 * (kt - 4 * qb) if kt - 4 * qb > 0 else 0) for kt in kts
                        ]
                        for g in range(3):
                            hh = 3 * t + g
                            ex = exs.pop((j, g))
                            for ii, kt in enumerate(kts):
                                off = offs[ii]
                                nc.tensor.matmul(
                                    avp[32 * g : 32 * g + 32, off:512],
                                    vl_sb[kt][:, 32 * hh : 32 * hh + 32],
                                    ex[:, 512 * ii + off : 512 * ii + 512],
                                    start=(kt == 0),
                                    stop=(kt == nkt - 1),
                                    tile_position=(0, 32 * g),
                                    skip_group_check=True,
                                )

                    emit_scores(0)
                    for j in range(1, npair):
                        emit_scores(j)
                        emit_av(j - 1)
                    emit_av(npair - 1)

                    nc.vector.tensor_copy(
                        alt_t[t][:, 512 * qb : 512 * qb + 512], avp[0:96, :]
                    )
                # prefetch next qb's vl tiles (PE filler while ACT runs)
                if qb < 3:
                    for i in range(4 * qb + 4, 4 * qb + 8):
                        vl_proj(i)

                # normalize + exchange this query block
                sl = slice(512 * qb, 512 * qb + 512)
                for tt in range(2):
                    smp = ps.tile([3, 512], f32, tag="pg", bufs=1, name="smp")
                    nc.tensor.matmul(
                        smp[:], eg_sb[:], alt_t[tt][:, sl], start=True, stop=True
                    )
                    rcp = work.tile([3, 512], f32r, tag="recip", bufs=2, name="rcp")
                    with nc.allow_low_precision(reason="recip feeds fp32r matmul"):
                        nc.vector.reciprocal(rcp[:], smp[:])
                    bcp = ps.tile([96, 512], f32, tag="pg", bufs=1, name="bcp")
                    nc.tensor.matmul(bcp[:], eb_sb[:], rcp[:], start=True, stop=True)
                    nc.vector.tensor_tensor(
                        alt_t[tt][:, sl], alt_t[tt][:, sl], bcp[:], op=MULT
                    )
                    nc.sync.dma_start(
                        out=cin[qb][96 * tt : 96 * tt + 96, :], in_=alt_t[tt][:, sl]
                    )
                nc.gpsimd.collective_compute(
                    "AllGather",
                    mybir.AluOpType.bypass,
                    replica_groups=rg,
                    ins=[cin[qb].opt()],
                    outs=[cout[qb].opt()],
                )

                # value-up projection for this query block (overlaps next qb)
                alf = [
                    act.tile([128, 512], f32r, tag="alf", bufs=6, name=f"alf{k}")
                    for k in range(3)
                ]
                for k in range(3):
                    nc.sync.dma_start(
                        out=alf[k][:], in_=cout[qb][128 * k : 128 * k + 128, :]
                    )
                for m in range(3):
                    pt = ps.tile([128, 512], f32, tag="pg", bufs=1, name="vup")
                    for k in range(3):
                        nc.tensor.matmul(
                            pt[:],
                            wvu_sb[k][:, 128 * m : 128 * m + 128],
                            alf[k][:],
                            start=(k == 0),
                            stop=(k == 2),
                        )
                    asb = work.tile([128, 512], f32, tag="aosb", bufs=2, name="asb")
                    nc.vector.tensor_copy(asb[:], pt[:])
                    nc.sync.dma_start(out=aot[128 * m : 128 * m + 128, sl], in_=asb[:])

            # ---- PE warm-keeper while the last collective/vu drains -------------
            wrm = ps.tile([128, 128], f32, tag="pg", bufs=1, name="wrm")
            for i in range(28):
                nc.tensor.matmul(
                    wrm[:],
                    idn_sb[:],
                    idn_sb[:],
                    start=(i == 0),
                    stop=(i == 27),
                )

            # ---- scramble (flat reinterpretation) + transposes ------------------
            aot_v = aot[:, :].rearrange("a b -> (a b)").rearrange(
                "(c d) -> c d", c=1024
            )
            aotT = [act.tile([128, 1024], f32r, name=f"aotT{j}") for j in range(6)]
            for u in range(8):
                at = work.tile([128, HID], f32, tag="at", bufs=2, name="at")
                nc.sync.dma_start(out=at[:], in_=aot_v[128 * u : 128 * u + 128, :])
                for j in range(6):
                    tp = ps.tile([128, 128], f32, tag="pg", bufs=1, name="tpp")
                    nc.tensor.transpose(tp[:], at[:, 128 * j : 128 * j + 128], idn_sb[:])
                    nc.vector.tensor_copy(aotT[j][:, 128 * u : 128 * u + 128], tp[:])

            # ---- output projection ---------------------------------------------
            wo_sb = load_w(WO, HID, "wo")
            for m in range(6):
                for nb in range(2):
                    pt = ps.tile([128, 512], f32, tag="pg", bufs=1, name="outp")
                    for k in range(6):
                        nc.tensor.matmul(
                            pt[:],
                            wo_sb[k][:, 128 * m : 128 * m + 128],
                            aotT[k][:, 512 * nb : 512 * nb + 512],
                            start=(k == 0),
                            stop=False,
                        )
                    nc.tensor.matmul(
                        pt[:],
                        wo_sb[6][:, 128 * m : 128 * m + 128],
                        ones_row[:, 512 * nb : 512 * nb + 512],
                        start=False,
                        stop=True,
                    )
                    osb = work.tile([128, 512], f32, tag="osb", bufs=2, name="osb")
                    nc.vector.tensor_copy(osb[:], pt[:])
                    nc.sync.dma_start(
                        out=OUTT[128 * m : 128 * m + 128, 512 * nb : 512 * nb + 512],
                        in_=osb[:],
                    )

    nc.finalize()
    _CACHE["nc"] = nc
    return nc


def _host_prep(inputs):
    x = np.asarray(inputs["x"], np.float32)
    Wq = np.asarray(inputs["Wq"], np.float32)
    bq = np.asarray(inputs["bq"], np.float32)
    Wkd = np.asarray(inputs["Wkd"], np.float32)
    bkd = np.asarray(inputs["bkd"], np.float32)
    Wvd = np.asarray(inputs["Wvd"], np.float32)
    bvd = np.asarray(inputs["bvd"], np.float32)
    Wvu = np.asarray(inputs["Wvu"], np.float32)
    bvu = np.asarray(inputs["bvu"], np.float32)
    Wo = np.asarray(inputs["Wo"], np.float32)
    bo = np.asarray(inputs["bo"], np.float32)

    mask = np.tril(np.ones((128, 128), np.float32)).T.astype(ml_dtypes.bfloat16)
    ident = np.eye(128, dtype=np.float32)
    eg = np.zeros((96, 3), np.float32)
    eb = np.zeros((3, 96), np.float32)
    for g in range(3):
        eg[32 * g + 16, g] = 1.0
        eb[g, 32 * g : 32 * g + 17] = 1.0
    ones = np.ones((1, 1024), np.float32)

    wo_pack = np.concatenate([Wo, bo[None, :]], axis=0)

    per_half = []
    for p in range(2):
        wq_pack = np.zeros((HID + 1, 256), np.float32)
        wk_pack = np.zeros((HID + 1, 256), np.float32)
        wv_pack = np.zeros((HID + 1, 256), np.float32)
        for hl in range(HL):
            hg = HL * p + hl
            t, g = hl // 3, hl % 3
            cols = slice(128 * t + 32 * g, 128 * t + 32 * g + 16)
            wq_pack[:HID, cols] = Wq[:, DH * hg : DH * hg + LD]
            wq_pack[HID, cols] = bq[DH * hg : DH * hg + LD]
            wk_pack[:HID, cols] = Wkd[:, LD * hg : LD * hg + LD]
            wk_pack[HID, cols] = bkd[LD * hg : LD * hg + LD]
            c0 = 32 * hl
            wv_pack[:HID, c0 : c0 + 16] = Wvd[:, LD * hg : LD * hg + LD]
            wv_pack[HID, c0 : c0 + 16] = bvd[LD * hg : LD * hg + LD]
            wv_pack[HID, c0 + 16] = 1.0
        wvu_pack = np.zeros((384, MHALF), np.float32)
        for hp in range(H):
            wvu_pack[32 * hp : 32 * hp + 16, :] = Wvu[
                LD * hp : LD * hp + LD, MHALF * p : MHALF * p + MHALF
            ]
        wvu_pack[16, :] = bvu[MHALF * p : MHALF * p + MHALF]
        per_half.append((wq_pack, wk_pack, wv_pack, wvu_pack))

    in_maps = []
    for c in range(N_CORES):
        b, p = c // 2, c % 2
        xt = np.concatenate(
            [np.ascontiguousarray(x[b].T), np.ones((1, S), np.float32)], axis=0
        )
        wq_pack, wk_pack, wv_pack, wvu_pack = per_half[p]
        in_maps.append(
            dict(
                xt=xt, wq=wq_pack, wk=wk_pack, wv=wv_pack, wvu=wvu_pack,
                wo=wo_pack, mask=mask, ident=ident, eg=eg, eb=eb, ones=ones,
            )
        )
    return in_maps


def _run(inputs, **kw):
    nc = _build_nc()
    in_maps = _host_prep(inputs)
    return run_bass_kernel_spmd(nc, in_maps, core_ids=list(range(N_CORES)), **kw)


def kernel(**inputs):
    res = _run(inputs)
    out = np.empty((B, S, HID), np.float32)
    for b in range(B):
        for p in range(2):
            out[b, 1024 * p : 1024 * p + 1024, :] = res.results[2 * b + p]["outt"].T
    return out

